# revision 31
# baseline (speedup 1.0000x reference)
"""FAVOR+ causal linear attention (relu kernel) on 8 TRN2 NeuronCores.

Problem: B=2, L=4096, H=8, D=64, M=128, fp32.
  qp = relu(q @ (P*ratio)^T) + 1e-3 ; kp likewise
  out_t = (sum_{j<=t} (qp_t . kp_j) v~_j) / den_t   (den via ones-column of v~)

Sharding: 16 (b,h) pairs -> 2 per core (embarrassingly parallel).
Host pre-transposes q/k to [pair, D, L] and pre-casts q/k/v/p to bf16 so all
device DMA is HWDGE.

Device per pair, super-chunk SC=512 (4 subs of 128), pairs interleaved:
  qpT = relu+1e-3 (DVE dual-op ts) ; kpT = ACT relu + Pool add-c
  kp natural via PE transpose of kpT -> ACT copy
  S^T diag blocks -> ACT copy -> Pool affine_select causal mask
  per sub s: delta_s = kp_s.T @ v~_s (independent closed PSUM groups)
             -> one batched DVE copy to SBUF
  num_s = st_s.T @ v~_s + qpT_s.T @ carry + sum_{j<s} qpT_s.T @ delta_j
  carry chain: ONE link per SC: carry += deltaSC (4-matmul PSUM group) via DVE TT
  epilogue per SC: batched reciprocal + broadcast-TT divide (DVE)
"""

import math

import numpy as np
import ml_dtypes

import concourse.bass as bass
import concourse.bacc as bacc
import concourse.mybir as mybir
import concourse.tile as tile
from concourse.bass_utils import run_bass_kernel_spmd
from concourse.masks import make_identity

F32 = mybir.dt.float32
BF16 = mybir.dt.bfloat16

B, L, H, D, M = 2, 4096, 8, 64, 128
NCORES = 8
NPAIR = (B * H) // NCORES  # 2 pairs per core
SC = 512                   # super-chunk timesteps
NSUB = SC // 128           # 4
NSC = L // SC              # 8
DV = D + 1                 # v augmented with ones column
STAB = 1e-3
RATIO = 1.0 / math.sqrt(M)

_NC_CACHE = {}


def build_nc():
    nc = bacc.Bacc("TRN2", target_bir_lowering=False, debug=False)
    qkT = nc.dram_tensor("qkT", [NPAIR, 2 * D, L], BF16, kind="ExternalInput").ap()
    v = nc.dram_tensor("v", [NPAIR, NSC, 128, NSUB, D], BF16, kind="ExternalInput").ap()
    pT = nc.dram_tensor("pT", [D, M], BF16, kind="ExternalInput").ap()
    out = nc.dram_tensor("out", [NPAIR, NSC, 128, NSUB, D], F32, kind="ExternalOutput").ap()

    with tile.TileContext(nc) as tc:
        with (
            tc.tile_pool(name="const", bufs=1) as cpool,
            tc.tile_pool(name="io", bufs=4) as iopool,
            tc.tile_pool(name="feat", bufs=3) as fpool,
            tc.tile_pool(name="state", bufs=4) as spool,
            tc.tile_pool(name="stats", bufs=6) as stpool,
            tc.tile_pool(name="ps_qpT", bufs=1, space="PSUM") as ps_qpT,
            tc.tile_pool(name="ps_kpT", bufs=1, space="PSUM") as ps_kpT,
            tc.tile_pool(name="ps_kp", bufs=1, space="PSUM") as ps_kp,
            tc.tile_pool(name="ps_st", bufs=1, space="PSUM") as ps_st,
            tc.tile_pool(name="ps_num", bufs=2, space="PSUM") as ps_num,
            tc.tile_pool(name="ps_d", bufs=1, space="PSUM") as ps_d,
            tc.tile_pool(name="ps_dsc", bufs=1, space="PSUM") as ps_dsc,
        ):
            # ---- constants ----
            ptile2 = cpool.tile([2 * D, M], BF16)  # PT stacked twice (base 0 / 64)
            nc.sync.dma_start(out=ptile2[0:D, :], in_=pT)
            nc.sync.dma_start(out=ptile2[D : 2 * D, :], in_=pT)
            ident = cpool.tile([128, 128], BF16)
            make_identity(nc, ident)

            kv_sb_t = [None] * NPAIR  # per-pair carry (SBUF bf16)
            for sc in range(NSC):
                for pair in range(NPAIR):
                    t0 = sc * SC
                    # ---- loads (HWDGE, bf16 in DRAM) ----
                    qkt = iopool.tile([2 * D, SC], BF16, tag="qkt", name=f"qkt_{pair}_{sc}")
                    nc.sync.dma_start(out=qkt, in_=qkT[pair, :, t0 : t0 + SC])
                    vt = iopool.tile([128, NSUB, DV], BF16, tag="vt", name=f"vt_{pair}_{sc}")
                    nc.gpsimd.memset(vt[:, :, D:DV], 1.0)
                    nc.sync.dma_start(out=vt[:, :, 0:D], in_=v[pair, sc])
                    # ---- feature projections ----
                    qpT_ps = ps_qpT.tile([M, SC], F32, tag="qpT_ps", name=f"qpT_ps_{pair}_{sc}")
                    nc.tensor.matmul(qpT_ps, ptile2[0:D, :], qkt[0:D, :], start=True, stop=True)
                    kpT_ps = ps_kpT.tile([M, SC], F32, tag="kpT_ps", name=f"kpT_ps_{pair}_{sc}")
                    nc.tensor.matmul(kpT_ps, ptile2[D : 2 * D, :], qkt[D : 2 * D, :], start=True, stop=True)
                    qpT = fpool.tile([M, SC], BF16, tag="qpT", name=f"qpT_{pair}_{sc}")
                    nc.vector.tensor_scalar(
                        qpT, qpT_ps, 0.0, STAB, mybir.AluOpType.max, mybir.AluOpType.add
                    )
                    kpT = fpool.tile([M, SC], BF16, tag="kpT", name=f"kpT_{pair}_{sc}")
                    nc.scalar.activation(kpT, kpT_ps, mybir.ActivationFunctionType.Relu)
                    nc.gpsimd.tensor_scalar_add(kpT, kpT, STAB)
                    # ---- kp natural layout via PE transpose ----
                    kp_ps = ps_kp.tile([128, NSUB * 128], BF16, tag="kp_ps", name=f"kp_ps_{pair}_{sc}")
                    for s in range(NSUB):
                        sl = slice(s * 128, (s + 1) * 128)
                        nc.tensor.transpose(kp_ps[:, sl], kpT[:, sl], ident)
                    kp = fpool.tile([128, NSUB * 128], BF16, tag="kp", name=f"kp_{pair}_{sc}")
                    nc.scalar.copy(out=kp, in_=kp_ps)
                    # ---- S^T diagonal blocks (masked) ----
                    st_ps = ps_st.tile([128, NSUB * 128], F32, tag="st_ps", name=f"st_ps_{pair}_{sc}")
                    for s in range(NSUB):
                        sl = slice(s * 128, (s + 1) * 128)
                        nc.tensor.matmul(st_ps[:, sl], kpT[:, sl], qpT[:, sl], start=True, stop=True)
                    st = fpool.tile([128, NSUB * 128], BF16, tag="st", name=f"st_{pair}_{sc}")
                    nc.scalar.copy(out=st, in_=st_ps)
                    nc.gpsimd.affine_select(
                        out=st, in_=st,
                        compare_op=mybir.AluOpType.is_ge, fill=0.0, base=0,
                        pattern=[[0, NSUB], [1, 128]], channel_multiplier=-1,
                    )
                    # ---- per-sub deltas (independent groups) + SC total ----
                    d_ps = ps_d.tile([128, NSUB, DV], F32, tag="d_ps", name=f"d_ps_{pair}_{sc}")
                    dsc_ps = ps_dsc.tile([M, DV], F32, tag="dsc_ps", name=f"dsc_ps_{pair}_{sc}")
                    for s in range(NSUB):
                        sl = slice(s * 128, (s + 1) * 128)
                        nc.tensor.matmul(d_ps[:, s, :], kp[:, sl], vt[:, s, :], start=True, stop=True)
                        nc.tensor.matmul(
                            dsc_ps, kp[:, sl], vt[:, s, :],
                            start=(s == 0), stop=(s == NSUB - 1),
                        )
                    d_sb = fpool.tile([128, NSUB, DV], BF16, tag="d_sb", name=f"d_sb_{pair}_{sc}")
                    nc.vector.tensor_copy(d_sb[:, 0:2, :], d_ps[:, 0:2, :])
                    nc.vector.tensor_copy(d_sb[:, 2:NSUB, :], d_ps[:, 2:NSUB, :])
                    # ---- num: intra diag + carry + delta routing ----
                    carry = kv_sb_t[pair]
                    num_ps = ps_num.tile([128, NSUB, DV], F32, tag="num_ps", name=f"num_ps_{pair}_{sc}")
                    out_sb = iopool.tile([128, NSUB, D], F32, tag="out_sb", name=f"out_sb_{pair}_{sc}")
                    for s in range(NSUB):
                        sl = slice(s * 128, (s + 1) * 128)
                        n_mm = 1 + (1 if carry is not None else 0) + s
                        mm = 1
                        nc.tensor.matmul(
                            num_ps[:, s, :], st[:, sl], vt[:, s, :],
                            start=True, stop=(mm == n_mm),
                        )
                        if carry is not None:
                            mm += 1
                            nc.tensor.matmul(
                                num_ps[:, s, :], qpT[:, sl], carry,
                                start=False, stop=(mm == n_mm),
                            )
                        for j in range(s):
                            mm += 1
                            nc.tensor.matmul(
                                num_ps[:, s, :], qpT[:, sl], d_sb[:, j, :],
                                start=False, stop=(mm == n_mm),
                            )
                    # ---- carry chain: one link per SC ----
                    if sc < NSC - 1:
                        kv_new = spool.tile([M, DV], BF16, tag="kv_sb", name=f"kv_{pair}_{sc}")
                        if carry is None:
                            nc.vector.tensor_copy(kv_new, dsc_ps)
                        else:
                            nc.vector.tensor_tensor(kv_new, dsc_ps, carry, mybir.AluOpType.add)
                        kv_sb_t[pair] = kv_new
                    # ---- epilogue: batched recip + broadcast divide ----
                    recip4 = stpool.tile([128, NSUB], F32, tag="recip4", name=f"recip_{pair}_{sc}")
                    nc.vector.reciprocal(recip4, num_ps[:, :, D])
                    recip_bcast = bass.AP(
                        tensor=recip4.tensor,
                        offset=recip4.offset,
                        ap=[recip4.ap[0], recip4.ap[1], [0, D]],
                    )
                    nc.vector.tensor_tensor(
                        out_sb, num_ps[:, :, 0:D], recip_bcast, mybir.AluOpType.mult
                    )
                    nc.sync.dma_start(out=out[pair, sc], in_=out_sb)
    nc.compile()
    return nc


def _get_nc():
    if "nc" not in _NC_CACHE:
        _NC_CACHE["nc"] = build_nc()
    return _NC_CACHE["nc"]


def shard_inputs(query, key, value, projection_matrix):
    """Full inputs -> per-core in_maps (host-side layout prep + bf16 cast)."""
    bf = ml_dtypes.bfloat16
    q = np.transpose(query, (0, 2, 3, 1)).reshape(B * H, D, L)
    k = np.transpose(key, (0, 2, 3, 1)).reshape(B * H, D, L)
    qk = np.concatenate([q, k], axis=1).astype(bf)  # [BH, 2D, L]
    vv = np.transpose(value, (0, 2, 1, 3)).reshape(B * H, NSC, NSUB, 128, D)
    vv = np.transpose(vv, (0, 1, 3, 2, 4)).astype(bf)  # [BH, NSC, 128, NSUB, D]
    pT = np.ascontiguousarray((projection_matrix * RATIO).T).astype(bf)
    in_maps = []
    for c in range(NCORES):
        sl = slice(c * NPAIR, (c + 1) * NPAIR)
        in_maps.append(
            {
                "qkT": np.ascontiguousarray(qk[sl]),
                "v": np.ascontiguousarray(vv[sl]),
                "pT": pT,
            }
        )
    return in_maps


def unshard_output(results):
    """Per-core {'out': [NPAIR, L, D]} -> full [B, L, H, D]."""
    o = np.concatenate([r["out"] for r in results], axis=0)  # [BH, NSC, 128, NSUB, D]
    o = o.transpose(0, 1, 3, 2, 4).reshape(B, H, L, D).transpose(0, 2, 1, 3)
    return np.ascontiguousarray(o).astype(np.float32)


def kernel(query, key, value, projection_matrix, _trace=False):
    nc = _get_nc()
    in_maps = shard_inputs(
        np.asarray(query, dtype=np.float32),
        np.asarray(key, dtype=np.float32),
        np.asarray(value, dtype=np.float32),
        np.asarray(projection_matrix, dtype=np.float32),
    )
    res = run_bass_kernel_spmd(nc, in_maps, core_ids=list(range(NCORES)), trace=_trace)
    out = unshard_output(res.results)
    if _trace:
        return out, res
    return out
